# revision 4
# baseline (speedup 1.0000x reference)
"""Trainium2 Bass kernel for nn_PredLayer (soft gather / one-hot scatter of
per-class ConvLSTM states).

Full-input contract: kernel(**inputs) takes the unsharded numpy inputs and
returns (gathered_h, gathered_c, updated_h, updated_c) matching reference().

Sharding: data-parallel over the batch axis (bs=16 -> 2 per core, 8 cores).
Per-core device kernel streams (128, 3072) f32 tiles (h on partitions,
w*oc chunked by 2):
  gather:  acc = sum_c w[b,c] * states[c,b]      (tensor_scalar + 3 STT FMAs)
  scatter: the mask softmax(logits*1e10) is an exact one-hot in f32, so
  updated[c,b] = new[b] if c == argmax(logits[b]) else states[c,b]. Stores
  are predicated DMAs (cond = per-b class register), so they depend only on
  the loads, never on compute -- loads and stores overlap fully.
All per-core divergence (weights/class index) is input data, so one SPMD
program serves all cores.
"""

import numpy as np

NCLS, BS, H, W, OC = 4, 16, 128, 128, 48
N_CORES = 8
BS_LOCAL = BS // N_CORES          # 2
F = W * OC                        # 6144 f32 per h-row
N_WCHUNK = 2
FCH = F // N_WCHUNK               # 3072
GATHER_BETA = 3.0
SCATTER_BETA = 1e10

_COMPILED = None
LAST_RESULTS = None


def _softmax_f32(z):
    z = np.asarray(z, dtype=np.float32)
    e = np.exp(z - z.max(axis=-1, keepdims=True))
    return (e / e.sum(axis=-1, keepdims=True)).astype(np.float32)


def _build_bass():
    import concourse.bacc as bacc
    import concourse.mybir as mybir
    import concourse.tile as tile

    dt = mybir.dt.float32
    it = mybir.dt.int32
    mult = mybir.AluOpType.mult
    add = mybir.AluOpType.add

    # Bacc (not Bass): its finalize() runs generate_event_semaphores, which
    # splits multi-sem waits — walrus rejects >1 sync wait per instruction.
    nc = bacc.Bacc(None, target_bir_lowering=False)
    sh = nc.dram_tensor("sh", [NCLS, BS_LOCAL, H, F], dt, kind="ExternalInput")
    sc = nc.dram_tensor("sc", [NCLS, BS_LOCAL, H, F], dt, kind="ExternalInput")
    nh = nc.dram_tensor("nh", [BS_LOCAL, H, F], dt, kind="ExternalInput")
    ncn = nc.dram_tensor("ncn", [BS_LOCAL, H, F], dt, kind="ExternalInput")
    wm = nc.dram_tensor("wm", [128, BS_LOCAL * NCLS], dt, kind="ExternalInput")
    ct = nc.dram_tensor("ct", [128, BS_LOCAL], it, kind="ExternalInput")
    gh = nc.dram_tensor("gh", [BS_LOCAL, H, F], dt, kind="ExternalOutput")
    gc = nc.dram_tensor("gc", [BS_LOCAL, H, F], dt, kind="ExternalOutput")
    uh = nc.dram_tensor("uh", [NCLS, BS_LOCAL, H, F], dt, kind="ExternalOutput")
    uc = nc.dram_tensor("uc", [NCLS, BS_LOCAL, H, F], dt, kind="ExternalOutput")

    with tile.TileContext(nc) as tc:
        with (
            tc.tile_pool(name="sp", bufs=8) as sp,
            tc.tile_pool(name="npool", bufs=3) as npool,
            tc.tile_pool(name="ap", bufs=2) as ap,
            tc.tile_pool(name="wp", bufs=1) as wp,
        ):
            wmt = wp.tile([128, BS_LOCAL * NCLS], dt, tag="wmt")
            nc.sync.dma_start(wmt[:], wm[:])
            ctt = wp.tile([128, BS_LOCAL], it, tag="ctt")
            nc.sync.dma_start(ctt[:], ct[:])
            cls_vals = []
            for b in range(BS_LOCAL):
                reg = nc.scalar.alloc_register(f"cls{b}")
                nc.scalar.reg_load(reg, ctt[0:1, b : b + 1])
                cls_vals.append(nc.scalar.snap(reg, min_val=0, max_val=NCLS - 1))
            for b in range(BS_LOCAL):
                for s_in, n_in, g_out, u_out in ((sh, nh, gh, uh), (sc, ncn, gc, uc)):
                    for wi in range(N_WCHUNK):
                        w0 = wi * FCH
                        ntile = npool.tile([128, FCH], dt, tag="ntile")
                        nc.sync.dma_start(ntile[:], n_in[b, :, w0 : w0 + FCH])
                        acc = ap.tile([128, FCH], dt, tag="acc")
                        for c in range(NCLS):
                            st = sp.tile([128, FCH], dt, tag="st")
                            nc.sync.dma_start(st[:], s_in[c, b, :, w0 : w0 + FCH])
                            w_ap = wmt[:, b * NCLS + c : b * NCLS + c + 1]
                            if c == 0:
                                nc.vector.tensor_scalar_mul(acc[:], st[:], w_ap)
                            else:
                                nc.vector.scalar_tensor_tensor(
                                    acc[:], st[:], w_ap, acc[:], mult, add
                                )
                            # one-hot scatter: exactly one of this pair fires.
                            # ntile-store first so the (usually-firing) st
                            # store's WAW wait lands on a skipped DMA.
                            nc.scalar.dma_start(
                                u_out[c, b, :, w0 : w0 + FCH],
                                ntile[:],
                                cond=(cls_vals[b] == c),
                                cond_hint=False,
                            )
                            nc.scalar.dma_start(
                                u_out[c, b, :, w0 : w0 + FCH],
                                st[:],
                                cond=(cls_vals[b] != c),
                                cond_hint=True,
                            )
                        nc.scalar.dma_start(g_out[b, :, w0 : w0 + FCH], acc[:])
    # run_bass_via_pjrt doesn't finalize; Bacc needs it for alloc_regs +
    # generate_event_semaphores before serialization.
    nc.finalize()
    return nc


def _get_compiled():
    global _COMPILED
    if _COMPILED is None:
        _COMPILED = _build_bass()
    return _COMPILED


def kernel(states_h, states_c, new_h, new_c, logits, _trace=False, **_trace_kwargs):
    global LAST_RESULTS
    from concourse.bass_utils import run_bass_kernel_spmd

    states_h = np.asarray(states_h, dtype=np.float32).reshape(NCLS, BS, H, F)
    states_c = np.asarray(states_c, dtype=np.float32).reshape(NCLS, BS, H, F)
    new_h = np.asarray(new_h, dtype=np.float32).reshape(BS, H, F)
    new_c = np.asarray(new_c, dtype=np.float32).reshape(BS, H, F)
    logits = np.asarray(logits, dtype=np.float32)

    w = _softmax_f32(logits * np.float32(GATHER_BETA))        # (bs, ncls)
    m = _softmax_f32(logits * np.float32(SCATTER_BETA))       # (bs, ncls) one-hot
    cls = m.argmax(axis=1).astype(np.int32)                   # (bs,)

    in_maps = []
    for k in range(N_CORES):
        b0 = k * BS_LOCAL
        bsl = slice(b0, b0 + BS_LOCAL)
        wvals = np.ascontiguousarray(w[bsl].reshape(-1))      # (BS_LOCAL*NCLS,)
        in_maps.append(
            {
                "sh": np.ascontiguousarray(states_h[:, bsl]),
                "sc": np.ascontiguousarray(states_c[:, bsl]),
                "nh": np.ascontiguousarray(new_h[bsl]),
                "ncn": np.ascontiguousarray(new_c[bsl]),
                "wm": np.ascontiguousarray(np.broadcast_to(wvals, (128, wvals.size))),
                "ct": np.ascontiguousarray(
                    np.broadcast_to(cls[bsl], (128, BS_LOCAL))
                ),
            }
        )

    nc = _get_compiled()
    res = run_bass_kernel_spmd(
        nc, in_maps, core_ids=list(range(N_CORES)), trace=_trace, **_trace_kwargs
    )
    LAST_RESULTS = res
    outs = res.results

    gathered_h = np.concatenate([outs[k]["gh"] for k in range(N_CORES)], axis=0)
    gathered_c = np.concatenate([outs[k]["gc"] for k in range(N_CORES)], axis=0)
    updated_h = np.concatenate([outs[k]["uh"] for k in range(N_CORES)], axis=1)
    updated_c = np.concatenate([outs[k]["uc"] for k in range(N_CORES)], axis=1)

    return (
        gathered_h.reshape(BS, H, W, OC),
        gathered_c.reshape(BS, H, W, OC),
        updated_h.reshape(NCLS, BS, H, W, OC),
        updated_c.reshape(NCLS, BS, H, W, OC),
    )


# revision 6
# speedup vs baseline: 1.0335x; 1.0335x over previous
"""Trainium2 Bass kernel for nn_PredLayer (soft gather / one-hot scatter of
per-class ConvLSTM states).

Full-input contract: kernel(**inputs) takes the unsharded numpy inputs and
returns (gathered_h, gathered_c, updated_h, updated_c) matching reference().

Sharding: data-parallel over the batch axis (bs=16 -> 2 per core, 8 cores).
Per-core device kernel streams (128, 3072) f32 tiles (h on partitions,
w*oc chunked by 2):
  gather:  acc = sum_c w[b,c] * states[c,b]      (tensor_scalar + 3 STT FMAs)
  scatter: the mask softmax(logits*1e10) is an exact one-hot in f32, so
  updated[c,b] = new[b] if c == argmax(logits[b]) else states[c,b]. Stores
  are predicated DMAs (cond = per-b class register), so they depend only on
  the loads, never on compute -- loads and stores overlap fully.
All per-core divergence (weights/class index) is input data, so one SPMD
program serves all cores.
"""

import numpy as np

NCLS, BS, H, W, OC = 4, 16, 128, 128, 48
N_CORES = 8
BS_LOCAL = BS // N_CORES          # 2
F = W * OC                        # 6144 f32 per h-row
N_WCHUNK = 2
FCH = F // N_WCHUNK               # 3072
GATHER_BETA = 3.0
SCATTER_BETA = 1e10

_COMPILED = None
LAST_RESULTS = None


def _softmax_f32(z):
    z = np.asarray(z, dtype=np.float32)
    e = np.exp(z - z.max(axis=-1, keepdims=True))
    return (e / e.sum(axis=-1, keepdims=True)).astype(np.float32)


def _build_bass():
    import concourse.bacc as bacc
    import concourse.mybir as mybir
    import concourse.tile as tile

    dt = mybir.dt.float32
    it = mybir.dt.int32
    mult = mybir.AluOpType.mult
    add = mybir.AluOpType.add

    # Bacc (not Bass): its finalize() runs generate_event_semaphores, which
    # splits multi-sem waits — walrus rejects >1 sync wait per instruction.
    nc = bacc.Bacc(None, target_bir_lowering=False)
    sh = nc.dram_tensor("sh", [NCLS, BS_LOCAL, H, F], dt, kind="ExternalInput")
    sc = nc.dram_tensor("sc", [NCLS, BS_LOCAL, H, F], dt, kind="ExternalInput")
    nh = nc.dram_tensor("nh", [BS_LOCAL, H, F], dt, kind="ExternalInput")
    ncn = nc.dram_tensor("ncn", [BS_LOCAL, H, F], dt, kind="ExternalInput")
    wm = nc.dram_tensor("wm", [128, BS_LOCAL * NCLS], dt, kind="ExternalInput")
    ct = nc.dram_tensor("ct", [128, BS_LOCAL], it, kind="ExternalInput")
    gh = nc.dram_tensor("gh", [BS_LOCAL, H, F], dt, kind="ExternalOutput")
    gc = nc.dram_tensor("gc", [BS_LOCAL, H, F], dt, kind="ExternalOutput")
    uh = nc.dram_tensor("uh", [NCLS, BS_LOCAL, H, F], dt, kind="ExternalOutput")
    uc = nc.dram_tensor("uc", [NCLS, BS_LOCAL, H, F], dt, kind="ExternalOutput")

    with tile.TileContext(nc) as tc:
        with (
            tc.tile_pool(name="sp", bufs=9) as sp,
            tc.tile_pool(name="npool", bufs=3) as npool,
            tc.tile_pool(name="ap", bufs=2) as ap,
            tc.tile_pool(name="wp", bufs=1) as wp,
        ):
            wmt = wp.tile([128, BS_LOCAL * NCLS], dt, tag="wmt")
            nc.sync.dma_start(wmt[:], wm[:])
            ctt = wp.tile([128, BS_LOCAL], it, tag="ctt")
            nc.sync.dma_start(ctt[:], ct[:])
            cls_vals = []
            for b in range(BS_LOCAL):
                reg = nc.scalar.alloc_register(f"cls{b}")
                nc.scalar.reg_load(reg, ctt[0:1, b : b + 1])
                cls_vals.append(nc.scalar.snap(reg, min_val=0, max_val=NCLS - 1))
            for b in range(BS_LOCAL):
                for s_in, n_in, g_out, u_out in ((sh, nh, gh, uh), (sc, ncn, gc, uc)):
                    for wi in range(N_WCHUNK):
                        w0 = wi * FCH
                        ntile = npool.tile([128, FCH], dt, tag="ntile")
                        nc.sync.dma_start(ntile[:], n_in[b, :, w0 : w0 + FCH])
                        # one-hot scatter, new-state side: issue all 4
                        # predicated ntile-stores up front so each st-store's
                        # WAW wait below lands on a descriptor that is long
                        # completed (1 fires) or skipped (3 instant).
                        for c in range(NCLS):
                            nc.scalar.dma_start(
                                u_out[c, b, :, w0 : w0 + FCH],
                                ntile[:],
                                cond=(cls_vals[b] == c),
                                cond_hint=False,
                            )
                        acc = ap.tile([128, FCH], dt, tag="acc")
                        for c in range(NCLS):
                            st = sp.tile([128, FCH], dt, tag="st")
                            nc.sync.dma_start(st[:], s_in[c, b, :, w0 : w0 + FCH])
                            nc.scalar.dma_start(
                                u_out[c, b, :, w0 : w0 + FCH],
                                st[:],
                                cond=(cls_vals[b] != c),
                                cond_hint=True,
                            )
                            w_ap = wmt[:, b * NCLS + c : b * NCLS + c + 1]
                            if c == 0:
                                nc.vector.tensor_scalar_mul(acc[:], st[:], w_ap)
                            else:
                                nc.vector.scalar_tensor_tensor(
                                    acc[:], st[:], w_ap, acc[:], mult, add
                                )
                        nc.scalar.dma_start(g_out[b, :, w0 : w0 + FCH], acc[:])
    # run_bass_via_pjrt doesn't finalize; Bacc needs it for alloc_regs +
    # generate_event_semaphores before serialization.
    nc.finalize()
    return nc


def _get_compiled():
    global _COMPILED
    if _COMPILED is None:
        _COMPILED = _build_bass()
    return _COMPILED


def kernel(states_h, states_c, new_h, new_c, logits, _trace=False, **_trace_kwargs):
    global LAST_RESULTS
    from concourse.bass_utils import run_bass_kernel_spmd

    states_h = np.asarray(states_h, dtype=np.float32).reshape(NCLS, BS, H, F)
    states_c = np.asarray(states_c, dtype=np.float32).reshape(NCLS, BS, H, F)
    new_h = np.asarray(new_h, dtype=np.float32).reshape(BS, H, F)
    new_c = np.asarray(new_c, dtype=np.float32).reshape(BS, H, F)
    logits = np.asarray(logits, dtype=np.float32)

    w = _softmax_f32(logits * np.float32(GATHER_BETA))        # (bs, ncls)
    m = _softmax_f32(logits * np.float32(SCATTER_BETA))       # (bs, ncls) one-hot
    cls = m.argmax(axis=1).astype(np.int32)                   # (bs,)

    in_maps = []
    for k in range(N_CORES):
        b0 = k * BS_LOCAL
        bsl = slice(b0, b0 + BS_LOCAL)
        wvals = np.ascontiguousarray(w[bsl].reshape(-1))      # (BS_LOCAL*NCLS,)
        in_maps.append(
            {
                "sh": np.ascontiguousarray(states_h[:, bsl]),
                "sc": np.ascontiguousarray(states_c[:, bsl]),
                "nh": np.ascontiguousarray(new_h[bsl]),
                "ncn": np.ascontiguousarray(new_c[bsl]),
                "wm": np.ascontiguousarray(np.broadcast_to(wvals, (128, wvals.size))),
                "ct": np.ascontiguousarray(
                    np.broadcast_to(cls[bsl], (128, BS_LOCAL))
                ),
            }
        )

    nc = _get_compiled()
    res = run_bass_kernel_spmd(
        nc, in_maps, core_ids=list(range(N_CORES)), trace=_trace, **_trace_kwargs
    )
    LAST_RESULTS = res
    outs = res.results

    gathered_h = np.concatenate([outs[k]["gh"] for k in range(N_CORES)], axis=0)
    gathered_c = np.concatenate([outs[k]["gc"] for k in range(N_CORES)], axis=0)
    updated_h = np.concatenate([outs[k]["uh"] for k in range(N_CORES)], axis=1)
    updated_c = np.concatenate([outs[k]["uc"] for k in range(N_CORES)], axis=1)

    return (
        gathered_h.reshape(BS, H, W, OC),
        gathered_c.reshape(BS, H, W, OC),
        updated_h.reshape(NCLS, BS, H, W, OC),
        updated_c.reshape(NCLS, BS, H, W, OC),
    )
